# revision 25
# baseline (speedup 1.0000x reference)
"""Trainium2 Bass kernel for nn_BasicBlock (binary-conv residual block).

Math (reference):
  h  = BN3( RPReLU1(BN1(bconv(sign(x), w1))) + x )
  out= BN4( RPReLU2(BN2(bconv(sign(h), w2))) + h )
with training-mode BN over the FULL batch (exact cross-device stats),
bconv = conv3x3(pad=1) with weights sign(w)*mean(|w|) per out-channel.

Strategy: data-parallel over batch on 8 NeuronCores (16 images/core).
 - fp8e4 +-1 activations/weights; conv = 9 shifted DoubleRow matmuls
   (each contracts both 128-channel halves) into PSUM; integer sums exact.
 - alpha (mean|w|) folded into BN1/BN2 affine; constant per-channel shifts
   absorbed by downstream BNs are dropped. 1/c3 folded into BN2 coefs
   (BN4 is per-channel-scale invariant; needs c3 > 0).
 - Engine separation: PSUM drains / residual adds / squares on DVE,
   prelu + BN3-sign + x-sign on ACT (scalar), collectives on gpsimd,
   so serial combine chains never queue behind conv bookkeeping.
 - BN sums/sumsqs come free from accum_out on the drain / add ops plus
   one tensor_tensor_reduce square per image (no bn_stats chains).
 - Conv outputs S stored fp16 (integer-exact). h stays fp32: SBUF-
   resident until the BN3 sign consumes it, swapped to DRAM for the
   branch-2 residual reload (sign must read fp32 h - fp16 h flips
   borderline signs and breaches the error gate).
 - Collectives emitted in input-readiness order so critical ones never
   queue behind non-critical ones on the CC core.
"""

import sys

import numpy as np

sys.path.insert(0, "/opt/trn_rl_repo")

from contextlib import ExitStack

import concourse.bacc as bacc
import concourse.bass as bass
import concourse.bass_utils as _bu
import concourse.mybir as mybir
import concourse.tile as tile
from concourse.masks import make_identity

dt = mybir.dt
AF = mybir.ActivationFunctionType
ALU = mybir.AluOpType
AX = mybir.AxisListType

C = 256
H = W = 28
PH = PW = 30
SP = PH * PW          # padded pixels / image
HW = H * W            # valid pixels / image
MARG = 32             # margin around padded free axis (shifts up to +-31)
EPS = 1e-5
NPAR = 12
PJ = dict(g1=0, b1=1, g2=2, b2=3, g3=4, b3=5, g4=6, b4=7,
          gamma1=8, beta1=9, gamma2=10, beta2=11)


def _off(d):
    kh, kw = d // 3, d % 3
    return (kh - 1) * PW + (kw - 1)


def build_nc(n_img, n_cores):
    nc = bacc.Bacc("TRN2", target_bir_lowering=False, num_devices=n_cores,
                   name="basicblock")
    x_d = nc.declare_dram_parameter("x", [n_img, C, H, W], dt.float32, isOutput=False)
    w1_d = nc.declare_dram_parameter("w1", [C, C, 3, 3], dt.float32, isOutput=False)
    w2_d = nc.declare_dram_parameter("w2", [C, C, 3, 3], dt.float32, isOutput=False)
    p_d = nc.declare_dram_parameter("pars", [NPAR, C], dt.float32, isOutput=False)
    o_d = nc.declare_dram_parameter("out", [n_img, C, H, W], dt.float32, isOutput=True)

    FREE = n_img * SP
    XBW = FREE + 2 * MARG
    NLOC = float(n_img * HW)
    NTOT = float(n_cores * n_img * HW)
    rg = [list(range(n_cores))]

    with ExitStack() as ctx:
        tc = ctx.enter_context(tile.TileContext(nc))
        sing = ctx.enter_context(tc.tile_pool(name="sing", bufs=1))
        xbp = ctx.enter_context(tc.tile_pool(name="xbp", bufs=1))
        wtp = ctx.enter_context(tc.tile_pool(name="wtp", bufs=2))
        wop = ctx.enter_context(tc.tile_pool(name="wop", bufs=1))
        spool = ctx.enter_context(tc.tile_pool(name="spool", bufs=2 * n_img))
        hbuf = ctx.enter_context(tc.tile_pool(name="hbuf", bufs=n_img + 2))
        chkp = ctx.enter_context(tc.tile_pool(name="chkp", bufs=6))
        outp = ctx.enter_context(tc.tile_pool(name="outp", bufs=3))
        tmpp = ctx.enter_context(tc.tile_pool(name="tmpp", bufs=5))
        stp = ctx.enter_context(tc.tile_pool(name="stp", bufs=1))
        psp = ctx.enter_context(tc.tile_pool(name="psp", bufs=8, space="PSUM"))
        dccp = ctx.enter_context(tc.tile_pool(name="dccp", bufs=1, space="DRAM"))
        dswp = ctx.enter_context(tc.tile_pool(name="dswp", bufs=2 * n_img, space="DRAM"))

        # ---- constants / params -------------------------------------------------
        ident = sing.tile([128, 128], dt.bfloat16, name="ident")
        make_identity(nc, ident)
        par = sing.tile([128, NPAR, 2], dt.float32, name="par")
        nc.sync.dma_start(out=par, in_=p_d[:, :].rearrange("j (h c) -> c j h", h=2))
        epst = sing.tile([128, 1], dt.float32, name="epst")
        nc.vector.memset(epst, EPS)
        # preload the ACT table family containing Sqrt (also serves
        # Sign/Prelu/Identity) so no table swap lands mid-pipeline
        warmsq = sing.tile([128, 1], dt.float32, name="warmsq")
        nc.scalar.activation(warmsq, epst, AF.Abs_reciprocal_sqrt, bias=epst)

        def P(j, ch):
            return par[:, PJ[j], ch:ch + 1]

        # ---- persistent big buffers --------------------------------------------
        # xb: [128, 2(k-half), XBW] fp8, DoubleRow-interleaved conv input.
        # Clear in ascending chunks split across DVE/gpsimd so the first
        # images' sign writes unblock almost immediately.
        # warmup collective first on the gpsimd queue: cc init (~45us) and
        # the first AllGather complete during startup, with no read-back DMA
        # clogging the sync queue.
        diw = dccp.tile([256], dt.float32, name="diw", tag="diw")
        dow = dccp.tile([n_cores * 256], dt.float32, name="dow", tag="dow")
        nc.sync.dma_start(out=diw.rearrange("(c f) -> c f", f=2), in_=par[:, 0, :])
        nc.gpsimd.collective_compute(
            "AllGather", ALU.bypass, replica_groups=rg, ins=[diw], outs=[dow])

        xbt = xbp.tile([128, 2, XBW], dt.float8e4, name="xbt", tag="xb")
        NMC = 8
        mcw = (XBW + NMC - 1) // NMC
        for k in range(NMC):
            lo = k * mcw
            hi = min(XBW, lo + mcw)
            nc.gpsimd.memset(xbt[:, :, lo:hi], 0.0)

        # wt: [128(i), 2(k-half), 9(tap), 256(o)] fp8 per conv
        wt = {cv: wtp.tile([128, 2, 9, C], dt.float8e4, name=f"wt{cv}", tag="wt")
              for cv in (1, 2)}

        def cf(name, w=1):
            return stp.tile([128, w], dt.float32, name=name, tag=name)

        # ---- weight prep (both convs, upfront) ---------------------------------
        # sign(w) on DVE; transposes on tensor engine ahead of all conv
        # matmuls in the PSUM rotation; wt copies on DVE.
        alpha = {1: cf("alpha1", 2), 2: cf("alpha2", 2)}

        def prep_w(cv, w_d):
            al = alpha[cv]
            for oh in (0, 1):
                wo = wop.tile([128, 2304], dt.float32, name=f"wo{cv}{oh}", tag="wo")
                nc.sync.dma_start(
                    out=wo,
                    in_=w_d[oh * 128:(oh + 1) * 128].rearrange("o i kh kw -> o (i kh kw)"))
                nc.vector.tensor_reduce(al[:, oh:oh + 1], wo, axis=AX.X, op=ALU.add,
                                        apply_absolute_value=True)
                ws = wop.tile([128, 2304], dt.bfloat16, name=f"ws{cv}{oh}", tag="ws")
                nc.vector.tensor_scalar(ws, wo, 0.0, None, ALU.is_ge)
                nc.vector.tensor_scalar(ws, ws, 2.0, -1.0, ALU.mult, ALU.add)
                wos = ws.rearrange("o (i k) -> o i k", k=9)
                for ih in (0, 1):
                    for k9 in range(9):
                        pt = psp.tile([128, 128], dt.bfloat16,
                                      name=f"tp{cv}{oh}{ih}{k9}", tag="ps")
                        nc.tensor.transpose(pt, wos[:, ih * 128:(ih + 1) * 128, k9],
                                            ident)
                        if cv == 1:
                            nc.vector.tensor_scalar_add(
                                wt[cv][:, ih, k9, oh * 128:(oh + 1) * 128], pt, 0.0)
                        else:
                            nc.scalar.copy(
                                wt[cv][:, ih, k9, oh * 128:(oh + 1) * 128], pt)
            nc.vector.tensor_scalar_mul(al, al, 1.0 / 2304.0)

        prep_w(1, w1_d)

        xsum = {ch: cf(f"xsum{ch}", n_img) for ch in (0, 1)}

        # ---- phase 0a: x -> sign(x) into padded fp8 buffer (all on ACT) --------
        # Sign the first few images upfront; the rest are injected into
        # conv1-m0's emission so scalar drains never queue behind them.
        sign_done = [0]

        def emit_xsigns(upto):
            while sign_done[0] < min(upto, n_img):
                im = sign_done[0]
                sign_done[0] += 1
                for ch in (0, 1):
                    xc = chkp.tile([128, HW], dt.float32, name=f"sx{ch}_{im}",
                                   tag="chk")
                    nc.sync.dma_start(
                        out=xc,
                        in_=x_d[im, ch * 128:(ch + 1) * 128]
                        .rearrange("c h w -> c (h w)"))
                    base = MARG + im * SP
                    dst = (xbt[:, ch, base:base + SP]
                           .rearrange("p (h w) -> p h w", w=PW)[:, 1:29, 1:29])
                    nc.scalar.activation(dst, xc.rearrange("p (h w) -> p h w", w=W),
                                         AF.Sign)
                    nc.vector.tensor_reduce(xsum[ch][:, im:im + 1], xc,
                                            axis=AX.X, op=ALU.add)

        emit_xsigns(6)

        # ---- conv macro ---------------------------------------------------------
        # DoubleRow fp8: one matmul contracts both 128-channel halves.
        # Weight-stationary: each (m, tap) weight serves a group of 8 psum
        # banks before switching. PSUM drains on DVE write fp16 S tiles and
        # emit the per-block channel sums via accum_out; one ttr square per
        # image emits the sumsq. No bn_stats.
        def conv_half(cv, m, S, st, inject=None):
            tiles = [(im, b) for im in range(n_img) for b in (0, 1)]
            n_grp = (len(tiles) + 3) // 4
            for im in range(n_img):
                S[(m, im)] = spool.tile([128, HW], dt.float16,
                                        name=f"S{cv}_{m}_{im}", tag="act")
            for gi, g0 in enumerate(range(0, len(tiles), 4)):
                grp = tiles[g0:g0 + 4]
                pts = {}
                for (im, b) in grp:
                    pts[(im, b)] = psp.tile([128, 450], dt.float32,
                                            name=f"cp{cv}_{m}_{im}_{b}",
                                            tag="ps")
                for d in range(9):
                    w_ap = wt[cv][:, :, d, m * 128:(m + 1) * 128]
                    for (im, b) in grp:
                        o = MARG + im * SP + b * 450 + _off(d)
                        nc.tensor.matmul(
                            pts[(im, b)], w_ap, xbt[:, :, o:o + 450],
                            perf_mode=mybir.MatmulPerfMode.DoubleRow,
                            start=(d == 0), stop=(d == 8))
                for (im, b) in grp:
                    pt = pts[(im, b)]
                    s_t = S[(m, im)]
                    pv = pt.rearrange("p (r c) -> p r c", c=PW)
                    sv = s_t.rearrange("p (r c) -> p r c", c=W)
                    r0 = 1 - b
                    nc.vector.tensor_scalar_add(sv[:, b * 14:(b + 1) * 14, :],
                                                pv[:, r0:r0 + 14, 1:29], 0.0)
                    if b == 1:
                        for q in (0, 1):
                            nc.vector.bn_stats(st[m][:, im, q],
                                               s_t[:, q * 392:(q + 1) * 392])
                if inject is not None:
                    inject((n_img * (gi + 1)) // n_grp)

        # ---- per-half stat helpers ---------------------------------------------
        def warm_cc(tag, key_tile):
            dw = dccp.tile([256], dt.float16, name=f"dw{tag}", tag=f"dw{tag}")
            ow = dccp.tile([n_cores * 256], dt.float16, name=f"ow{tag}",
                           tag=f"ow{tag}")
            nc.sync.dma_start(out=dw.rearrange("(c f) -> c f", f=2),
                              in_=key_tile[:, 0:2])
            nc.gpsimd.collective_compute(
                "AllGather", ALU.bypass, replica_groups=rg, ins=[dw], outs=[ow])

        def gtree(acc, out):
            # in-place halving tree over [128, n_img] -> out [128,1]; gpsimd
            r = n_img
            while r > 1:
                h = r // 2
                nc.gpsimd.tensor_add(acc[:, 0:h], acc[:, 0:h], acc[:, h:r])
                r = h
            nc.gpsimd.tensor_scalar_add(out, acc[:, 0:1], 0.0)

        def half_sums_bn(stm, tag):
            # stm: [128, n_img, 2, 6] bn_stats rows -> s2 [128,2] = (sum, sumsq)
            mv = cf(f"mv{tag}", 2)
            nc.vector.bn_aggr(mv, stm.rearrange("p a b s -> p (a b) s"))
            s2 = cf(f"s2{tag}", 2)
            nc.gpsimd.tensor_scalar_mul(s2[:, 0:1], mv[:, 0:1], NLOC)
            t0 = cf(f"t0{tag}")
            nc.gpsimd.tensor_mul(t0, mv[:, 0:1], mv[:, 0:1])
            nc.gpsimd.tensor_add(t0, t0, mv[:, 1:2])
            nc.gpsimd.tensor_scalar_mul(s2[:, 1:2], t0, NLOC)
            return s2

        def half_sums(parts, sqt, tag):
            # parts: per-image sum tiles to add elementwise; sqt: per-image
            # sumsq tile -> s2 [128,2] = (sum, sumsq); all on gpsimd
            s2 = cf(f"s2{tag}", 2)
            acc = cf(f"acc{tag}", n_img)
            if len(parts) == 1:
                nc.gpsimd.tensor_scalar_add(acc, parts[0], 0.0)
            else:
                nc.gpsimd.tensor_add(acc, parts[0], parts[1])
                for extra in parts[2:]:
                    nc.gpsimd.tensor_add(acc, acc, extra)
            gtree(acc, s2[:, 0:1])
            qac = cf(f"qac{tag}", n_img)
            nc.gpsimd.tensor_scalar_add(qac, sqt, 0.0)
            gtree(qac, s2[:, 1:2])
            return s2

        def ag_reduce(s2, tag):
            # AllGather the per-core [128,2] (sum,sumsq) half-stats; add locally.
            di = dccp.tile([256], dt.float32, name=f"di{tag}", tag=f"di{tag}")
            do = dccp.tile([n_cores * 256], dt.float32, name=f"do{tag}",
                           tag=f"do{tag}")
            nc.sync.dma_start(out=di.rearrange("(c f) -> c f", f=2), in_=s2)
            nc.gpsimd.collective_compute(
                "AllGather", ALU.bypass, replica_groups=rg, ins=[di], outs=[do])
            g8 = cf(f"g8{tag}", 2 * n_cores)
            nc.sync.dma_start(
                out=g8.rearrange("p (f r) -> p f r", f=2),
                in_=do.rearrange("(r c f) -> c f r", c=128, f=2))
            gv = g8.rearrange("p (f r) -> p f r", f=2)
            r = n_cores
            while r > 1:
                h = r // 2
                nc.gpsimd.tensor_add(gv[:, :, 0:h], gv[:, :, 0:h], gv[:, :, h:r])
                r = h
            g2 = cf(f"g2{tag}", 2)
            nc.gpsimd.tensor_scalar_add(g2, gv[:, :, 0], 0.0)
            return g2

        def ag_reduce2(s2a, s2b, tag):
            # one AllGather carrying two (sum,sumsq) pairs; gpsimd tree reduce
            di = dccp.tile([512], dt.float32, name=f"di{tag}", tag=f"di{tag}")
            do = dccp.tile([n_cores * 512], dt.float32, name=f"do{tag}",
                           tag=f"do{tag}")
            dv = di.rearrange("(c f) -> c f", f=4)
            nc.sync.dma_start(out=dv[:, 0:2], in_=s2a)
            nc.sync.dma_start(out=dv[:, 2:4], in_=s2b)
            nc.gpsimd.collective_compute(
                "AllGather", ALU.bypass, replica_groups=rg, ins=[di], outs=[do])
            g8 = cf(f"g8{tag}", 4 * n_cores)
            nc.sync.dma_start(
                out=g8.rearrange("p (f r) -> p f r", f=4),
                in_=do.rearrange("(r c f) -> c f r", c=128, f=4))
            gv = g8.rearrange("p (f r) -> p f r", f=4)
            r = n_cores
            while r > 1:
                h = r // 2
                nc.gpsimd.tensor_add(gv[:, :, 0:h], gv[:, :, 0:h], gv[:, :, h:r])
                r = h
            g2a = cf(f"g2a{tag}", 2)
            g2b = cf(f"g2b{tag}", 2)
            nc.gpsimd.tensor_scalar_add(g2a, gv[:, 0:2, 0], 0.0)
            nc.gpsimd.tensor_scalar_add(g2b, gv[:, 2:4, 0], 0.0)
            return g2a, g2b

        def mean_var(g2, tag):
            mean = cf(f"mean{tag}")
            var = cf(f"var{tag}")
            msq = cf(f"msq{tag}")
            nc.gpsimd.tensor_scalar_mul(mean, g2[:, 0:1], 1.0 / NTOT)
            nc.gpsimd.tensor_scalar_mul(var, g2[:, 1:2], 1.0 / NTOT)
            nc.gpsimd.tensor_mul(msq, mean, mean)
            nc.gpsimd.tensor_sub(var, var, msq)
            return mean, var

        def inv_of(var, jg, ch, tag):
            # g / sqrt(var + eps)
            ir = cf(f"ir{tag}")
            nc.scalar.activation(ir, var, AF.Abs_reciprocal_sqrt, bias=epst)
            inv = cf(f"inv{tag}")
            nc.gpsimd.tensor_mul(inv, ir, P(jg, ch))
            return inv

        def bn_conv_coefs(cv, g2, ch, jg, jb, jgam, tag):
            # y = alpha*S: c=alpha*inv, dg=b-alpha*mean*inv-gamma
            mean, var = mean_var(g2, tag)
            al = alpha[cv][:, ch:ch + 1]
            a2 = cf(f"a2{tag}")
            nc.gpsimd.tensor_mul(a2, al, al)
            vy = cf(f"vy{tag}")
            nc.gpsimd.tensor_mul(vy, var, a2)
            inv = inv_of(vy, jg, ch, tag)
            c = cf(f"c{tag}")
            nc.gpsimd.tensor_mul(c, al, inv)
            my = cf(f"my{tag}")
            nc.gpsimd.tensor_mul(my, mean, al)
            nc.gpsimd.tensor_mul(my, my, inv)
            dg = cf(f"dg{tag}")
            nc.gpsimd.tensor_sub(dg, P(jb, ch), my)
            nc.gpsimd.tensor_sub(dg, dg, P(jgam, ch))
            return c, dg

        def bn_plain_coefs(g2, ch, jg, jb, tag):
            # c = g*inv, d = b - mean*c
            mean, var = mean_var(g2, tag)
            inv = inv_of(var, jg, ch, tag)
            d = cf(f"d{tag}")
            nc.gpsimd.tensor_mul(mean, mean, inv)
            nc.gpsimd.tensor_sub(d, P(jb, ch), mean)
            return inv, d

        # ---- combine chunk emitters --------------------------------------------
        # prelu on ACT; residual add + square on DVE with accum stats.
        # h fp32 lives in hbuf until the BN3 sign reads it; swapped to DRAM
        # for the branch-2 residual.
        HCUR = {}
        HSW = {}

        def make_combiner1(ch, S1, tsum, hssq, coefs):
            done = [0]

            def emit(upto):
                c1, d1g = coefs()
                while done[0] < min(upto, n_img):
                    im = done[0]
                    done[0] += 1
                    s_t = S1[(ch, im)]
                    xc = chkp.tile([128, HW], dt.float32, name=f"xc{ch}_{im}",
                                   tag="chk")
                    nc.sync.dma_start(
                        out=xc,
                        in_=x_d[im, ch * 128:(ch + 1) * 128]
                        .rearrange("c h w -> c (h w)"))
                    t = tmpp.tile([128, HW], dt.float32, name=f"t1_{ch}_{im}",
                                  tag="t")
                    nc.scalar.activation(t, s_t, AF.Prelu, bias=d1g, scale=c1,
                                         alpha=P("beta1", ch),
                                         accum_out=tsum[ch][:, im:im + 1])
                    h = hbuf.tile([128, HW], dt.float32, name=f"h{ch}_{im}",
                                  tag="h")
                    HCUR[(ch, im)] = h
                    nc.vector.tensor_add(h, t, xc)
                    if ch == 0:
                        hq = outp.tile([128, HW], dt.float32,
                                       name=f"hq{ch}_{im}", tag="oc")
                        nc.scalar.activation(hq, h, AF.Square,
                                             accum_out=hssq[0][:, im:im + 1])
                    else:
                        for q in (0, 1):
                            nc.vector.bn_stats(sth1[:, im, q],
                                               h[:, q * 392:(q + 1) * 392])
                    dr = dswp.tile([128, HW], dt.float32, name=f"hs{ch}_{im}",
                                   tag="swap")
                    HSW[(ch, im)] = dr
                    nc.sync.dma_start(out=dr, in_=h)
            return emit

        def emit_signs(ch, cc3):
            # conv2 input: sign(c3*h + d3) into xbt (needs c3 > 0); h fp32
            c3, d3 = cc3[ch]
            for im in range(n_img):
                h = HCUR[(ch, im)]
                base = MARG + im * SP
                dst = (xbt[:, ch, base:base + SP]
                       .rearrange("p (h w) -> p h w", w=PW)[:, 1:29, 1:29])
                nc.scalar.activation(dst, h.rearrange("p (h w) -> p h w", w=W),
                                     AF.Sign, bias=d3, scale=c3)

        # ---- conv1 + interleaved ch0 pipeline ----------------------------------
        st1 = {m: stp.tile([128, n_img, 2, 6], dt.float32, name=f"st1_{m}",
                           tag=f"st1_{m}") for m in (0, 1)}
        sth1 = stp.tile([128, n_img, 2, 6], dt.float32, name="sth1", tag="sth1")
        stf1 = stp.tile([128, n_img, 2, 6], dt.float32, name="stf1", tag="stf1")
        tsum = {ch: cf(f"tsum{ch}", n_img) for ch in (0, 1)}
        hssq = {0: cf("hssq0", n_img)}
        S1 = {}
        cc3 = {}

        conv_half(1, 0, S1, st1, inject=lambda upto: emit_xsigns(upto + 6))
        warm_cc("a", S1[(0, 10 % n_img)])


        g2_10 = ag_reduce(half_sums_bn(st1[0], "b10"), "b10")
        cc1_0 = bn_conv_coefs(1, g2_10, 0, "g1", "b1", "gamma1", "b10")
        comb0 = make_combiner1(0, S1, tsum, hssq, lambda: cc1_0)
        conv_half(1, 1, S1, st1, inject=comb0)
        comb0(n_img)
        warm_cc("b", S1[(1, 4 % n_img)])

        # ch1's BN1 chain first: ch0's combine may spill past conv1-m1, and
        # the ch1 chain (combine + BN3 + sign) is the long pole before conv2.
        g2_11 = ag_reduce(half_sums_bn(st1[1], "b11"), "b11")
        cc1_1 = bn_conv_coefs(1, g2_11, 1, "g1", "b1", "gamma1", "b11")
        comb1 = make_combiner1(1, S1, tsum, hssq, lambda: cc1_1)
        comb1(n_img)

        g2h0 = ag_reduce(half_sums([tsum[0], xsum[0]], hssq[0], "b30"), "b30")
        cc3[0] = bn_plain_coefs(g2h0, 0, "g3", "b3", "b30")
        emit_signs(0, cc3)
        prep_w(2, w2_d)

        g2h1 = ag_reduce(half_sums_bn(sth1, "b31"), "b31")
        cc3[1] = bn_plain_coefs(g2h1, 1, "g3", "b3", "b31")
        emit_signs(1, cc3)

        # ---- conv2 + interleaved ch0 pipeline ----------------------------------
        # BN2 coefs are divided by c3 so combine2 computes h2s/c3 = prelu2/c3
        # + h'_raw; BN4 then normalizes away the 1/c3 exactly (needs c3 > 0).
        def bn2_coefs(ch, tag):
            g2 = ag_reduce(half_sums_bn(st2[ch], tag), tag)
            c2, d2g = bn_conv_coefs(2, g2, ch, "g2", "b2", "gamma2", tag)
            c3sq = cf(f"c3sq{ch}")
            nc.gpsimd.tensor_mul(c3sq, cc3[ch][0], cc3[ch][0])
            r3 = cf(f"r3{ch}")
            nc.scalar.activation(r3, c3sq, AF.Abs_reciprocal_sqrt)
            nc.gpsimd.tensor_mul(c2, c2, r3)
            nc.gpsimd.tensor_mul(d2g, d2g, r3)
            return c2, d2g

        def make_combiner2(ch, S2, t2sum, fssq, coefs):
            done = [0]

            def emit(upto):
                c2, d2g = coefs()
                while done[0] < min(upto, n_img):
                    im = done[0]
                    done[0] += 1
                    s2t = S2[(ch, im)]
                    hc = chkp.tile([128, HW], dt.float32, name=f"hc{ch}_{im}",
                                   tag="chk")
                    nc.sync.dma_start(out=hc, in_=HSW[(ch, im)])
                    t2 = tmpp.tile([128, HW], dt.float32, name=f"t2_{ch}_{im}",
                                   tag="t")
                    nc.scalar.activation(t2, s2t, AF.Prelu, bias=d2g, scale=c2,
                                         alpha=P("beta2", ch),
                                         accum_out=t2sum[ch][:, im:im + 1])
                    nc.vector.tensor_add(s2t, t2, hc)
                    if ch == 0:
                        fq = outp.tile([128, HW], dt.float32,
                                       name=f"fq{ch}_{im}", tag="oc")
                        nc.scalar.activation(fq, s2t, AF.Square,
                                             accum_out=fssq[0][:, im:im + 1])
                    else:
                        for q in (0, 1):
                            nc.vector.bn_stats(stf1[:, im, q],
                                               s2t[:, q * 392:(q + 1) * 392])
            return emit

        def bn4_out(ch, tag):
            if ch == 0:
                g2f = ag_reduce(half_sums([t2sum[0], tsum[0], xsum[0]],
                                          fssq[0], tag), tag)
            else:
                g2f = ag_reduce(half_sums_bn(stf1, tag), tag)
            c4, d4 = bn_plain_coefs(g2f, ch, "g4", "b4", tag)
            for im in range(n_img):
                s2t = S2[(ch, im)]
                oc = outp.tile([128, HW], dt.float32, name=f"oc{ch}_{im}",
                               tag="oc")
                nc.vector.tensor_scalar(oc, s2t, c4, d4, ALU.mult, ALU.add)
                nc.sync.dma_start(
                    out=o_d[im, ch * 128:(ch + 1) * 128].rearrange("c h w -> c (h w)"),
                    in_=oc)

        st2 = {m: stp.tile([128, n_img, 2, 6], dt.float32, name=f"st2_{m}",
                           tag=f"st2_{m}") for m in (0, 1)}
        t2sum = {ch: cf(f"t2sum{ch}", n_img) for ch in (0, 1)}
        fssq = {0: cf("fssq0", n_img)}
        S2 = {}

        conv_half(2, 0, S2, st2)
        warm_cc("c", S2[(0, 10 % n_img)])
        c2d2_0 = bn2_coefs(0, "b20")
        comb2_0 = make_combiner2(0, S2, t2sum, fssq, lambda: c2d2_0)
        conv_half(2, 1, S2, st2, inject=comb2_0)
        comb2_0(n_img)
        warm_cc("d", S2[(1, 4 % n_img)])

        # ch1's chain first: its BN2 stats are ready at conv2-m1 end, while
        # ch0's BN4 stats lag behind the vector backlog; triggers must be
        # input-readiness ordered or the CC queue head-of-line blocks.
        c2d2_1 = bn2_coefs(1, "b21")
        comb2_1 = make_combiner2(1, S2, t2sum, fssq, lambda: c2d2_1)
        comb2_1(n_img)
        bn4_out(0, "b40")
        bn4_out(1, "b41")

    nc.compile()
    return nc


_NC_CACHE = {}


def get_nc(n_img, n_cores):
    key = (n_img, n_cores)
    if key not in _NC_CACHE:
        _NC_CACHE[key] = build_nc(n_img, n_cores)
    return _NC_CACHE[key]


def pack_pars(inputs):
    return np.stack([np.asarray(inputs[k], np.float32) for k in
                     ["g1", "b1", "g2", "b2", "g3", "b3", "g4", "b4",
                      "gamma1", "beta1", "gamma2", "beta2"]])


def kernel(**inputs):
    from concourse.bass_utils import run_bass_kernel_spmd

    x = np.asarray(inputs["x"], np.float32)
    n_cores = 8
    n_img = x.shape[0] // n_cores
    nc = get_nc(n_img, n_cores)
    pars = pack_pars(inputs)
    w1 = np.asarray(inputs["w1"], np.float32)
    w2 = np.asarray(inputs["w2"], np.float32)
    in_maps = [
        {"x": np.ascontiguousarray(x[c * n_img:(c + 1) * n_img]),
         "w1": w1, "w2": w2, "pars": pars}
        for c in range(n_cores)
    ]
    res = run_bass_kernel_spmd(nc, in_maps, core_ids=list(range(n_cores)))
    return np.concatenate([res.results[c]["out"] for c in range(n_cores)], axis=0)


if __name__ == "__main__":
    nc = build_nc(2, 2)
    print("built ok")


# revision 26
# speedup vs baseline: 1.0718x; 1.0718x over previous
"""Trainium2 Bass kernel for nn_BasicBlock (binary-conv residual block).

Math (reference):
  h  = BN3( RPReLU1(BN1(bconv(sign(x), w1))) + x )
  out= BN4( RPReLU2(BN2(bconv(sign(h), w2))) + h )
with training-mode BN over the FULL batch (exact cross-device stats),
bconv = conv3x3(pad=1) with weights sign(w)*mean(|w|) per out-channel.

Strategy: data-parallel over batch on 8 NeuronCores (16 images/core).
 - fp8e4 +-1 activations/weights; conv = 9 shifted DoubleRow matmuls
   (each contracts both 128-channel halves) into PSUM; integer sums exact.
 - alpha (mean|w|) folded into BN1/BN2 affine; constant per-channel shifts
   absorbed by downstream BNs are dropped. 1/c3 folded into BN2 coefs
   (BN4 is per-channel-scale invariant; needs c3 > 0).
 - Engine separation: PSUM drains / residual adds / squares on DVE,
   prelu + BN3-sign + x-sign on ACT (scalar), collectives on gpsimd,
   so serial combine chains never queue behind conv bookkeeping.
 - BN sums/sumsqs come free from accum_out on the drain / add ops plus
   one tensor_tensor_reduce square per image (no bn_stats chains).
 - Conv outputs S stored fp16 (integer-exact). h stays fp32: SBUF-
   resident until the BN3 sign consumes it, swapped to DRAM for the
   branch-2 residual reload (sign must read fp32 h - fp16 h flips
   borderline signs and breaches the error gate).
 - Collectives emitted in input-readiness order so critical ones never
   queue behind non-critical ones on the CC core.
"""

import sys

import numpy as np

sys.path.insert(0, "/opt/trn_rl_repo")

from contextlib import ExitStack

import concourse.bacc as bacc
import concourse.bass as bass
import concourse.bass_utils as _bu
import concourse.mybir as mybir
import concourse.tile as tile
from concourse.masks import make_identity

dt = mybir.dt
AF = mybir.ActivationFunctionType
ALU = mybir.AluOpType
AX = mybir.AxisListType

C = 256
H = W = 28
PH = PW = 30
SP = PH * PW          # padded pixels / image
HW = H * W            # valid pixels / image
MARG = 32             # margin around padded free axis (shifts up to +-31)
EPS = 1e-5
NPAR = 12
PJ = dict(g1=0, b1=1, g2=2, b2=3, g3=4, b3=5, g4=6, b4=7,
          gamma1=8, beta1=9, gamma2=10, beta2=11)


def _off(d):
    kh, kw = d // 3, d % 3
    return (kh - 1) * PW + (kw - 1)


def build_nc(n_img, n_cores):
    nc = bacc.Bacc("TRN2", target_bir_lowering=False, num_devices=n_cores,
                   name="basicblock")
    x_d = nc.declare_dram_parameter("x", [n_img, C, H, W], dt.float32, isOutput=False)
    w1_d = nc.declare_dram_parameter("w1", [C, C, 3, 3], dt.float32, isOutput=False)
    w2_d = nc.declare_dram_parameter("w2", [C, C, 3, 3], dt.float32, isOutput=False)
    p_d = nc.declare_dram_parameter("pars", [NPAR, C], dt.float32, isOutput=False)
    o_d = nc.declare_dram_parameter("out", [n_img, C, H, W], dt.float32, isOutput=True)

    FREE = n_img * SP
    XBW = FREE + 2 * MARG
    NLOC = float(n_img * HW)
    NTOT = float(n_cores * n_img * HW)
    rg = [list(range(n_cores))]

    with ExitStack() as ctx:
        tc = ctx.enter_context(tile.TileContext(nc))
        sing = ctx.enter_context(tc.tile_pool(name="sing", bufs=1))
        xbp = ctx.enter_context(tc.tile_pool(name="xbp", bufs=1))
        wtp = ctx.enter_context(tc.tile_pool(name="wtp", bufs=2))
        wop = ctx.enter_context(tc.tile_pool(name="wop", bufs=1))
        spool = ctx.enter_context(tc.tile_pool(name="spool", bufs=2 * n_img))
        hbuf = ctx.enter_context(tc.tile_pool(name="hbuf", bufs=n_img + 2))
        chkp = ctx.enter_context(tc.tile_pool(name="chkp", bufs=6))
        outp = ctx.enter_context(tc.tile_pool(name="outp", bufs=3))
        tmpp = ctx.enter_context(tc.tile_pool(name="tmpp", bufs=5))
        stp = ctx.enter_context(tc.tile_pool(name="stp", bufs=1))
        psp = ctx.enter_context(tc.tile_pool(name="psp", bufs=8, space="PSUM"))
        dccp = ctx.enter_context(tc.tile_pool(name="dccp", bufs=1, space="DRAM"))
        dswp = ctx.enter_context(tc.tile_pool(name="dswp", bufs=2 * n_img, space="DRAM"))

        # ---- constants / params -------------------------------------------------
        ident = sing.tile([128, 128], dt.bfloat16, name="ident")
        make_identity(nc, ident)
        par = sing.tile([128, NPAR, 2], dt.float32, name="par")
        nc.sync.dma_start(out=par, in_=p_d[:, :].rearrange("j (h c) -> c j h", h=2))
        epst = sing.tile([128, 1], dt.float32, name="epst")
        nc.vector.memset(epst, EPS)
        # preload the ACT table family containing Sqrt (also serves
        # Sign/Prelu/Identity) so no table swap lands mid-pipeline
        warmsq = sing.tile([128, 1], dt.float32, name="warmsq")
        nc.scalar.activation(warmsq, epst, AF.Abs_reciprocal_sqrt, bias=epst)

        def P(j, ch):
            return par[:, PJ[j], ch:ch + 1]

        # ---- persistent big buffers --------------------------------------------
        # xb: [128, 2(k-half), XBW] fp8, DoubleRow-interleaved conv input.
        # Clear in ascending chunks split across DVE/gpsimd so the first
        # images' sign writes unblock almost immediately.
        # warmup collective first on the gpsimd queue: cc init (~45us) and
        # the first AllGather complete during startup, with no read-back DMA
        # clogging the sync queue.
        diw = dccp.tile([256], dt.float32, name="diw", tag="diw")
        dow = dccp.tile([n_cores * 256], dt.float32, name="dow", tag="dow")
        nc.sync.dma_start(out=diw.rearrange("(c f) -> c f", f=2), in_=par[:, 0, :])
        nc.gpsimd.collective_compute(
            "AllGather", ALU.bypass, replica_groups=rg, ins=[diw], outs=[dow])

        xbt = xbp.tile([128, 2, XBW], dt.float8e4, name="xbt", tag="xb")
        NMC = 8
        mcw = (XBW + NMC - 1) // NMC
        for k in range(NMC):
            lo = k * mcw
            hi = min(XBW, lo + mcw)
            nc.gpsimd.memset(xbt[:, :, lo:hi], 0.0)

        # wt: [128(i), 2(k-half), 9(tap), 256(o)] fp8 per conv
        wt = {cv: wtp.tile([128, 2, 9, C], dt.float8e4, name=f"wt{cv}", tag="wt")
              for cv in (1, 2)}

        def cf(name, w=1):
            return stp.tile([128, w], dt.float32, name=name, tag=name)

        # ---- weight prep (both convs, upfront) ---------------------------------
        # sign(w) on DVE; transposes on tensor engine ahead of all conv
        # matmuls in the PSUM rotation; wt copies on DVE.
        alpha = {1: cf("alpha1", 2), 2: cf("alpha2", 2)}

        def prep_w(cv, w_d):
            al = alpha[cv]
            for oh in (0, 1):
                wo = wop.tile([128, 2304], dt.float32, name=f"wo{cv}{oh}", tag="wo")
                nc.sync.dma_start(
                    out=wo,
                    in_=w_d[oh * 128:(oh + 1) * 128].rearrange("o i kh kw -> o (i kh kw)"))
                nc.vector.tensor_reduce(al[:, oh:oh + 1], wo, axis=AX.X, op=ALU.add,
                                        apply_absolute_value=True)
                ws = wop.tile([128, 2304], dt.bfloat16, name=f"ws{cv}{oh}", tag="ws")
                nc.vector.tensor_scalar(ws, wo, 0.0, None, ALU.is_ge)
                nc.vector.tensor_scalar(ws, ws, 2.0, -1.0, ALU.mult, ALU.add)
                wos = ws.rearrange("o (i k) -> o i k", k=9)
                for ih in (0, 1):
                    for k9 in range(9):
                        pt = psp.tile([128, 128], dt.bfloat16,
                                      name=f"tp{cv}{oh}{ih}{k9}", tag="ps")
                        nc.tensor.transpose(pt, wos[:, ih * 128:(ih + 1) * 128, k9],
                                            ident)
                        if cv == 1:
                            nc.vector.tensor_scalar_add(
                                wt[cv][:, ih, k9, oh * 128:(oh + 1) * 128], pt, 0.0)
                        else:
                            nc.scalar.copy(
                                wt[cv][:, ih, k9, oh * 128:(oh + 1) * 128], pt)
            nc.vector.tensor_scalar_mul(al, al, 1.0 / 2304.0)

        prep_w(1, w1_d)

        xsum = {ch: cf(f"xsum{ch}", n_img) for ch in (0, 1)}

        # ---- phase 0a: x -> sign(x) into padded fp8 buffer (all on ACT) --------
        # Sign the first few images upfront; the rest are injected into
        # conv1-m0's emission so scalar drains never queue behind them.
        sign_done = [0]

        def emit_xsigns(upto):
            while sign_done[0] < min(upto, n_img):
                im = sign_done[0]
                sign_done[0] += 1
                for ch in (0, 1):
                    xc = chkp.tile([128, HW], dt.float32, name=f"sx{ch}_{im}",
                                   tag="chk")
                    nc.sync.dma_start(
                        out=xc,
                        in_=x_d[im, ch * 128:(ch + 1) * 128]
                        .rearrange("c h w -> c (h w)"))
                    base = MARG + im * SP
                    dst = (xbt[:, ch, base:base + SP]
                           .rearrange("p (h w) -> p h w", w=PW)[:, 1:29, 1:29])
                    nc.scalar.activation(dst, xc.rearrange("p (h w) -> p h w", w=W),
                                         AF.Sign)
                    nc.vector.tensor_reduce(xsum[ch][:, im:im + 1], xc,
                                            axis=AX.X, op=ALU.add)

        emit_xsigns(6)

        # ---- conv macro ---------------------------------------------------------
        # DoubleRow fp8: one matmul contracts both 128-channel halves.
        # Weight-stationary: each (m, tap) weight serves a group of 8 psum
        # banks before switching. PSUM drains on DVE write fp16 S tiles and
        # emit the per-block channel sums via accum_out; one ttr square per
        # image emits the sumsq. No bn_stats.
        def conv_half(cv, m, S, st, inject=None):
            tiles = [(im, b) for im in range(n_img) for b in (0, 1)]
            n_grp = (len(tiles) + 3) // 4
            for im in range(n_img):
                S[(m, im)] = spool.tile([128, HW], dt.float16,
                                        name=f"S{cv}_{m}_{im}", tag="act")
            for gi, g0 in enumerate(range(0, len(tiles), 4)):
                grp = tiles[g0:g0 + 4]
                pts = {}
                for (im, b) in grp:
                    pts[(im, b)] = psp.tile([128, 450], dt.float32,
                                            name=f"cp{cv}_{m}_{im}_{b}",
                                            tag="ps")
                for d in range(9):
                    w_ap = wt[cv][:, :, d, m * 128:(m + 1) * 128]
                    for (im, b) in grp:
                        o = MARG + im * SP + b * 450 + _off(d)
                        nc.tensor.matmul(
                            pts[(im, b)], w_ap, xbt[:, :, o:o + 450],
                            perf_mode=mybir.MatmulPerfMode.DoubleRow,
                            start=(d == 0), stop=(d == 8))
                for (im, b) in grp:
                    pt = pts[(im, b)]
                    s_t = S[(m, im)]
                    pv = pt.rearrange("p (r c) -> p r c", c=PW)
                    sv = s_t.rearrange("p (r c) -> p r c", c=W)
                    r0 = 1 - b
                    nc.vector.tensor_scalar_add(sv[:, b * 14:(b + 1) * 14, :],
                                                pv[:, r0:r0 + 14, 1:29], 0.0)
                    if b == 1:
                        for q in (0, 1):
                            nc.vector.bn_stats(st[m][:, im, q],
                                               s_t[:, q * 392:(q + 1) * 392])
                if inject is not None:
                    inject((n_img * (gi + 1)) // n_grp)

        # ---- per-half stat helpers ---------------------------------------------
        def warm_cc(tag, key_tile):
            dw = dccp.tile([256], dt.float16, name=f"dw{tag}", tag=f"dw{tag}")
            ow = dccp.tile([n_cores * 256], dt.float16, name=f"ow{tag}",
                           tag=f"ow{tag}")
            nc.sync.dma_start(out=dw.rearrange("(c f) -> c f", f=2),
                              in_=key_tile[:, 0:2])
            nc.gpsimd.collective_compute(
                "AllGather", ALU.bypass, replica_groups=rg, ins=[dw], outs=[ow])

        def gtree(acc, out):
            # in-place halving tree over [128, n_img] -> out [128,1]; gpsimd
            r = n_img
            while r > 1:
                h = r // 2
                nc.gpsimd.tensor_add(acc[:, 0:h], acc[:, 0:h], acc[:, h:r])
                r = h
            nc.gpsimd.tensor_scalar_add(out, acc[:, 0:1], 0.0)

        def half_sums_bn(stm, tag):
            # stm: [128, n_img, 2, 6] bn_stats rows -> s2 [128,2] = (sum, sumsq)
            mv = cf(f"mv{tag}", 2)
            nc.vector.bn_aggr(mv, stm.rearrange("p a b s -> p (a b) s"))
            s2 = cf(f"s2{tag}", 2)
            nc.gpsimd.tensor_scalar_mul(s2[:, 0:1], mv[:, 0:1], NLOC)
            t0 = cf(f"t0{tag}")
            nc.gpsimd.tensor_mul(t0, mv[:, 0:1], mv[:, 0:1])
            nc.gpsimd.tensor_add(t0, t0, mv[:, 1:2])
            nc.gpsimd.tensor_scalar_mul(s2[:, 1:2], t0, NLOC)
            return s2

        def half_sums(parts, sqt, tag):
            # parts: per-image sum tiles to add elementwise; sqt: per-image
            # sumsq tile -> s2 [128,2] = (sum, sumsq); all on gpsimd
            s2 = cf(f"s2{tag}", 2)
            acc = cf(f"acc{tag}", n_img)
            if len(parts) == 1:
                nc.gpsimd.tensor_scalar_add(acc, parts[0], 0.0)
            else:
                nc.gpsimd.tensor_add(acc, parts[0], parts[1])
                for extra in parts[2:]:
                    nc.gpsimd.tensor_add(acc, acc, extra)
            gtree(acc, s2[:, 0:1])
            qac = cf(f"qac{tag}", n_img)
            nc.gpsimd.tensor_scalar_add(qac, sqt, 0.0)
            gtree(qac, s2[:, 1:2])
            return s2

        def ag_reduce(s2, tag):
            # AllGather the per-core [128,2] (sum,sumsq) half-stats; add locally.
            di = dccp.tile([256], dt.float32, name=f"di{tag}", tag=f"di{tag}")
            do = dccp.tile([n_cores * 256], dt.float32, name=f"do{tag}",
                           tag=f"do{tag}")
            nc.sync.dma_start(out=di.rearrange("(c f) -> c f", f=2), in_=s2)
            nc.gpsimd.collective_compute(
                "AllGather", ALU.bypass, replica_groups=rg, ins=[di], outs=[do])
            g8 = cf(f"g8{tag}", 2 * n_cores)
            nc.sync.dma_start(
                out=g8.rearrange("p (f r) -> p f r", f=2),
                in_=do.rearrange("(r c f) -> c f r", c=128, f=2))
            gv = g8.rearrange("p (f r) -> p f r", f=2)
            r = n_cores
            while r > 1:
                h = r // 2
                nc.gpsimd.tensor_add(gv[:, :, 0:h], gv[:, :, 0:h], gv[:, :, h:r])
                r = h
            g2 = cf(f"g2{tag}", 2)
            nc.gpsimd.tensor_scalar_add(g2, gv[:, :, 0], 0.0)
            return g2

        def ag_reduce2(s2a, s2b, tag):
            # one AllGather carrying two (sum,sumsq) pairs; gpsimd tree reduce
            di = dccp.tile([512], dt.float32, name=f"di{tag}", tag=f"di{tag}")
            do = dccp.tile([n_cores * 512], dt.float32, name=f"do{tag}",
                           tag=f"do{tag}")
            dv = di.rearrange("(c f) -> c f", f=4)
            nc.sync.dma_start(out=dv[:, 0:2], in_=s2a)
            nc.sync.dma_start(out=dv[:, 2:4], in_=s2b)
            nc.gpsimd.collective_compute(
                "AllGather", ALU.bypass, replica_groups=rg, ins=[di], outs=[do])
            g8 = cf(f"g8{tag}", 4 * n_cores)
            nc.sync.dma_start(
                out=g8.rearrange("p (f r) -> p f r", f=4),
                in_=do.rearrange("(r c f) -> c f r", c=128, f=4))
            gv = g8.rearrange("p (f r) -> p f r", f=4)
            r = n_cores
            while r > 1:
                h = r // 2
                nc.gpsimd.tensor_add(gv[:, :, 0:h], gv[:, :, 0:h], gv[:, :, h:r])
                r = h
            g2a = cf(f"g2a{tag}", 2)
            g2b = cf(f"g2b{tag}", 2)
            nc.gpsimd.tensor_scalar_add(g2a, gv[:, 0:2, 0], 0.0)
            nc.gpsimd.tensor_scalar_add(g2b, gv[:, 2:4, 0], 0.0)
            return g2a, g2b

        def mean_var(g2, tag):
            mean = cf(f"mean{tag}")
            var = cf(f"var{tag}")
            msq = cf(f"msq{tag}")
            nc.gpsimd.tensor_scalar_mul(mean, g2[:, 0:1], 1.0 / NTOT)
            nc.gpsimd.tensor_scalar_mul(var, g2[:, 1:2], 1.0 / NTOT)
            nc.gpsimd.tensor_mul(msq, mean, mean)
            nc.gpsimd.tensor_sub(var, var, msq)
            return mean, var

        def inv_of(var, jg, ch, tag):
            # g / sqrt(var + eps)
            ir = cf(f"ir{tag}")
            nc.scalar.activation(ir, var, AF.Abs_reciprocal_sqrt, bias=epst)
            inv = cf(f"inv{tag}")
            nc.gpsimd.tensor_mul(inv, ir, P(jg, ch))
            return inv

        def bn_conv_coefs(cv, g2, ch, jg, jb, jgam, tag):
            # y = alpha*S: c=alpha*inv, dg=b-alpha*mean*inv-gamma
            mean, var = mean_var(g2, tag)
            al = alpha[cv][:, ch:ch + 1]
            a2 = cf(f"a2{tag}")
            nc.gpsimd.tensor_mul(a2, al, al)
            vy = cf(f"vy{tag}")
            nc.gpsimd.tensor_mul(vy, var, a2)
            inv = inv_of(vy, jg, ch, tag)
            c = cf(f"c{tag}")
            nc.gpsimd.tensor_mul(c, al, inv)
            my = cf(f"my{tag}")
            nc.gpsimd.tensor_mul(my, mean, al)
            nc.gpsimd.tensor_mul(my, my, inv)
            dg = cf(f"dg{tag}")
            nc.gpsimd.tensor_sub(dg, P(jb, ch), my)
            nc.gpsimd.tensor_sub(dg, dg, P(jgam, ch))
            return c, dg

        def bn_plain_coefs(g2, ch, jg, jb, tag):
            # c = g*inv, d = b - mean*c
            mean, var = mean_var(g2, tag)
            inv = inv_of(var, jg, ch, tag)
            d = cf(f"d{tag}")
            nc.gpsimd.tensor_mul(mean, mean, inv)
            nc.gpsimd.tensor_sub(d, P(jb, ch), mean)
            return inv, d

        # ---- combine chunk emitters --------------------------------------------
        # prelu on ACT; residual add + square on DVE with accum stats.
        # h fp32 lives in hbuf until the BN3 sign reads it; swapped to DRAM
        # for the branch-2 residual.
        HCUR = {}
        HSW = {}

        def make_combiner1(ch, S1, tsum, hssq, coefs):
            done = [0]

            def emit(upto):
                c1, d1g = coefs()
                while done[0] < min(upto, n_img):
                    im = done[0]
                    done[0] += 1
                    s_t = S1[(ch, im)]
                    xc = chkp.tile([128, HW], dt.float32, name=f"xc{ch}_{im}",
                                   tag="chk")
                    nc.sync.dma_start(
                        out=xc,
                        in_=x_d[im, ch * 128:(ch + 1) * 128]
                        .rearrange("c h w -> c (h w)"))
                    t = tmpp.tile([128, HW], dt.float32, name=f"t1_{ch}_{im}",
                                  tag="t")
                    nc.scalar.activation(t, s_t, AF.Prelu, bias=d1g, scale=c1,
                                         alpha=P("beta1", ch),
                                         accum_out=tsum[ch][:, im:im + 1])
                    h = hbuf.tile([128, HW], dt.float32, name=f"h{ch}_{im}",
                                  tag="h")
                    HCUR[(ch, im)] = h
                    nc.vector.tensor_add(h, t, xc)
                    if ch == 0:
                        hq = outp.tile([128, HW], dt.float32,
                                       name=f"hq{ch}_{im}", tag="oc")
                        nc.scalar.activation(hq, h, AF.Square,
                                             accum_out=hssq[0][:, im:im + 1])
                    else:
                        for q in (0, 1):
                            nc.vector.bn_stats(sth1[:, im, q],
                                               h[:, q * 392:(q + 1) * 392])
                    dr = dswp.tile([128, HW], dt.float32, name=f"hs{ch}_{im}",
                                   tag="swap")
                    HSW[(ch, im)] = dr
                    nc.sync.dma_start(out=dr, in_=h)
            return emit

        def emit_signs(ch, cc3):
            # conv2 input: sign(c3*h + d3) into xbt (needs c3 > 0); h fp32
            c3, d3 = cc3[ch]
            for im in range(n_img):
                h = HCUR[(ch, im)]
                base = MARG + im * SP
                dst = (xbt[:, ch, base:base + SP]
                       .rearrange("p (h w) -> p h w", w=PW)[:, 1:29, 1:29])
                nc.scalar.activation(dst, h.rearrange("p (h w) -> p h w", w=W),
                                     AF.Sign, bias=d3, scale=c3)

        # ---- conv1 + interleaved ch0 pipeline ----------------------------------
        st1 = {m: stp.tile([128, n_img, 2, 6], dt.float32, name=f"st1_{m}",
                           tag=f"st1_{m}") for m in (0, 1)}
        sth1 = stp.tile([128, n_img, 2, 6], dt.float32, name="sth1", tag="sth1")
        stf1 = stp.tile([128, n_img, 2, 6], dt.float32, name="stf1", tag="stf1")
        tsum = {ch: cf(f"tsum{ch}", n_img) for ch in (0, 1)}
        hssq = {0: cf("hssq0", n_img)}
        S1 = {}
        cc3 = {}

        conv_half(1, 0, S1, st1, inject=lambda upto: emit_xsigns(upto + 6))


        g2_10 = ag_reduce(half_sums_bn(st1[0], "b10"), "b10")
        cc1_0 = bn_conv_coefs(1, g2_10, 0, "g1", "b1", "gamma1", "b10")
        comb0 = make_combiner1(0, S1, tsum, hssq, lambda: cc1_0)
        conv_half(1, 1, S1, st1, inject=comb0)
        comb0(n_img)

        # ch1's BN1 chain first: ch0's combine may spill past conv1-m1, and
        # the ch1 chain (combine + BN3 + sign) is the long pole before conv2.
        g2_11 = ag_reduce(half_sums_bn(st1[1], "b11"), "b11")
        cc1_1 = bn_conv_coefs(1, g2_11, 1, "g1", "b1", "gamma1", "b11")
        comb1 = make_combiner1(1, S1, tsum, hssq, lambda: cc1_1)
        comb1(n_img)

        g2h0 = ag_reduce(half_sums([tsum[0], xsum[0]], hssq[0], "b30"), "b30")
        cc3[0] = bn_plain_coefs(g2h0, 0, "g3", "b3", "b30")
        emit_signs(0, cc3)
        prep_w(2, w2_d)

        g2h1 = ag_reduce(half_sums_bn(sth1, "b31"), "b31")
        cc3[1] = bn_plain_coefs(g2h1, 1, "g3", "b3", "b31")
        emit_signs(1, cc3)

        # ---- conv2 + interleaved ch0 pipeline ----------------------------------
        # BN2 coefs are divided by c3 so combine2 computes h2s/c3 = prelu2/c3
        # + h'_raw; BN4 then normalizes away the 1/c3 exactly (needs c3 > 0).
        def bn2_coefs(ch, tag):
            g2 = ag_reduce(half_sums_bn(st2[ch], tag), tag)
            c2, d2g = bn_conv_coefs(2, g2, ch, "g2", "b2", "gamma2", tag)
            c3sq = cf(f"c3sq{ch}")
            nc.gpsimd.tensor_mul(c3sq, cc3[ch][0], cc3[ch][0])
            r3 = cf(f"r3{ch}")
            nc.scalar.activation(r3, c3sq, AF.Abs_reciprocal_sqrt)
            nc.gpsimd.tensor_mul(c2, c2, r3)
            nc.gpsimd.tensor_mul(d2g, d2g, r3)
            return c2, d2g

        def make_combiner2(ch, S2, t2sum, fssq, coefs):
            done = [0]

            def emit(upto):
                c2, d2g = coefs()
                while done[0] < min(upto, n_img):
                    im = done[0]
                    done[0] += 1
                    s2t = S2[(ch, im)]
                    hc = chkp.tile([128, HW], dt.float32, name=f"hc{ch}_{im}",
                                   tag="chk")
                    nc.sync.dma_start(out=hc, in_=HSW[(ch, im)])
                    t2 = tmpp.tile([128, HW], dt.float32, name=f"t2_{ch}_{im}",
                                   tag="t")
                    nc.scalar.activation(t2, s2t, AF.Prelu, bias=d2g, scale=c2,
                                         alpha=P("beta2", ch),
                                         accum_out=t2sum[ch][:, im:im + 1])
                    nc.vector.tensor_add(s2t, t2, hc)
                    if ch == 0:
                        fq = outp.tile([128, HW], dt.float32,
                                       name=f"fq{ch}_{im}", tag="oc")
                        nc.scalar.activation(fq, s2t, AF.Square,
                                             accum_out=fssq[0][:, im:im + 1])
                    else:
                        for q in (0, 1):
                            nc.vector.bn_stats(stf1[:, im, q],
                                               s2t[:, q * 392:(q + 1) * 392])
            return emit

        def bn4_out(ch, tag):
            if ch == 0:
                g2f = ag_reduce(half_sums([t2sum[0], tsum[0], xsum[0]],
                                          fssq[0], tag), tag)
            else:
                g2f = ag_reduce(half_sums_bn(stf1, tag), tag)
            c4, d4 = bn_plain_coefs(g2f, ch, "g4", "b4", tag)
            for im in range(n_img):
                s2t = S2[(ch, im)]
                oc = outp.tile([128, HW], dt.float32, name=f"oc{ch}_{im}",
                               tag="oc")
                nc.vector.tensor_scalar(oc, s2t, c4, d4, ALU.mult, ALU.add)
                nc.sync.dma_start(
                    out=o_d[im, ch * 128:(ch + 1) * 128].rearrange("c h w -> c (h w)"),
                    in_=oc)

        st2 = {m: stp.tile([128, n_img, 2, 6], dt.float32, name=f"st2_{m}",
                           tag=f"st2_{m}") for m in (0, 1)}
        t2sum = {ch: cf(f"t2sum{ch}", n_img) for ch in (0, 1)}
        fssq = {0: cf("fssq0", n_img)}
        S2 = {}

        conv_half(2, 0, S2, st2)
        c2d2_0 = bn2_coefs(0, "b20")
        comb2_0 = make_combiner2(0, S2, t2sum, fssq, lambda: c2d2_0)
        conv_half(2, 1, S2, st2, inject=comb2_0)
        comb2_0(n_img)

        # ch1's chain first: its BN2 stats are ready at conv2-m1 end, while
        # ch0's BN4 stats lag behind the vector backlog; triggers must be
        # input-readiness ordered or the CC queue head-of-line blocks.
        c2d2_1 = bn2_coefs(1, "b21")
        comb2_1 = make_combiner2(1, S2, t2sum, fssq, lambda: c2d2_1)
        comb2_1(n_img)
        bn4_out(0, "b40")
        bn4_out(1, "b41")

    nc.compile()
    return nc


_NC_CACHE = {}


def get_nc(n_img, n_cores):
    key = (n_img, n_cores)
    if key not in _NC_CACHE:
        _NC_CACHE[key] = build_nc(n_img, n_cores)
    return _NC_CACHE[key]


def pack_pars(inputs):
    return np.stack([np.asarray(inputs[k], np.float32) for k in
                     ["g1", "b1", "g2", "b2", "g3", "b3", "g4", "b4",
                      "gamma1", "beta1", "gamma2", "beta2"]])


def kernel(**inputs):
    from concourse.bass_utils import run_bass_kernel_spmd

    x = np.asarray(inputs["x"], np.float32)
    n_cores = 8
    n_img = x.shape[0] // n_cores
    nc = get_nc(n_img, n_cores)
    pars = pack_pars(inputs)
    w1 = np.asarray(inputs["w1"], np.float32)
    w2 = np.asarray(inputs["w2"], np.float32)
    in_maps = [
        {"x": np.ascontiguousarray(x[c * n_img:(c + 1) * n_img]),
         "w1": w1, "w2": w2, "pars": pars}
        for c in range(n_cores)
    ]
    res = run_bass_kernel_spmd(nc, in_maps, core_ids=list(range(n_cores)))
    return np.concatenate([res.results[c]["out"] for c in range(n_cores)], axis=0)


if __name__ == "__main__":
    nc = build_nc(2, 2)
    print("built ok")
